# revision 19
# baseline (speedup 1.0000x reference)
"""ChordAwareAttention Trainium2 kernel.

Sharding: 16 heads across 8 cores (2 heads/core), both batches on every core
(4 (b,h) pairs per core). Inputs are host-transposed to d-major [H, B*S] so all
projections run on-device; scores are computed transposed ST[k,q] so the
softmax denominator falls out of the ctx matmul (ones column in v_aug) and ctx
needs no on-chip transpose. attn is written to DRAM as [k,q] bf16 per pair and
returned to the caller as a zero-copy transposed fp32 view.
"""
import sys

try:
    import concourse.bass as bass  # noqa: F401
except ImportError:
    sys.path.insert(0, "/opt/trn_rl_repo")

import numpy as np
import ml_dtypes

import concourse.bass as bass
import concourse.mybir as mybir
import concourse.tile as tile
from concourse import bacc, bass_utils
from contextlib import ExitStack

F32 = mybir.dt.float32
BF16 = mybir.dt.bfloat16
BF = ml_dtypes.bfloat16

B, H, NH, HD = 2, 1024, 16, 64
NCORES = 8
HPC = NH // NCORES          # heads per core
SCALE = (HD ** -0.5) / 5.0  # applied inside exp

_programs = {}


def _emit(nc, S, repeat):
    BS = B * S
    SCW = min(512, S)        # phase-1 seq chunk width
    NSC = BS // SCW
    NKT = S // 128           # 128-row k tiles per pair
    QCH = min(512, S)        # phase-2 q chunk width
    NQC = S // QCH           # q chunks per pair
    NDK = H // 128           # 8 din chunks

    qT = nc.dram_tensor("qT", [H, BS], BF16, kind="ExternalInput").ap()
    kT = nc.dram_tensor("kT", [H, BS], BF16, kind="ExternalInput").ap()
    vT = nc.dram_tensor("vT", [H, BS], BF16, kind="ExternalInput").ap()
    pitch2 = nc.dram_tensor("pitch2", [12, S], BF16, kind="ExternalInput").ap()
    melbass = nc.dram_tensor("melbass", [36, S], BF16, kind="ExternalInput").ap()
    rhy2 = nc.dram_tensor("rhy2", [4, S], BF16, kind="ExternalInput").ap()

    wq = nc.dram_tensor("wq", [HPC, 3, H, 128], BF16, kind="ExternalInput").ap()
    bq = nc.dram_tensor("bq", [HPC, 3, 128, 1], F32, kind="ExternalInput").ap()
    wkey = nc.dram_tensor("wkey", [HPC, H, 128], BF16, kind="ExternalInput").ap()
    wch = nc.dram_tensor("wch", [HPC, 12, 128], BF16, kind="ExternalInput").ap()
    wmb = nc.dram_tensor("wmb", [HPC, 36, 128], BF16, kind="ExternalInput").ap()
    wrhy = nc.dram_tensor("wrhy", [HPC, 4, 64], BF16, kind="ExternalInput").ap()
    bk0 = nc.dram_tensor("bk0", [HPC, 128, 1], F32, kind="ExternalInput").ap()
    bk1 = nc.dram_tensor("bk1", [HPC, 128, 1], F32, kind="ExternalInput").ap()
    bk2 = nc.dram_tensor("bk2", [HPC, 64, 1], F32, kind="ExternalInput").ap()
    wv = nc.dram_tensor("wv", [H, 128], BF16, kind="ExternalInput").ap()
    wout = nc.dram_tensor("wout", [HPC, 64, H], BF16, kind="ExternalInput").ap()

    attnT = nc.dram_tensor("attnT", [2 * HPC, S, S], BF16, kind="ExternalOutput").ap()
    outp = nc.dram_tensor("outp", [BS, H], BF16, kind="ExternalOutput").ap()

    qT_r = qT.rearrange("(dk p) s -> p dk s", p=128)
    kT_r = kT.rearrange("(dk p) s -> p dk s", p=128)
    vT_r = vT.rearrange("(dk p) s -> p dk s", p=128)

    with tile.TileContext(nc) as tc, ExitStack() as top:
        singles = top.enter_context(tc.tile_pool(name="singles", bufs=1))
        qc_sb = [[singles.tile([128, BS], BF16, tag=f"qc{hl}{c}", name=f"qc{hl}{c}")
                  for c in range(3)] for hl in range(HPC)]
        kc0_sb = [singles.tile([128, BS], BF16, tag=f"kc0{hl}", name=f"kc0{hl}") for hl in range(HPC)]
        kc1_sb = [singles.tile([128, S], BF16, tag=f"kc1{hl}", name=f"kc1{hl}") for hl in range(HPC)]
        kc2_sb = [singles.tile([128, S], BF16, tag=f"kc2{hl}", name=f"kc2{hl}") for hl in range(HPC)]
        v_aug = [singles.tile([128, NKT, 65], BF16, tag=f"vaug{p}", name=f"vaug{p}")
                 for p in range(2 * HPC)]
        ctx_sb = [singles.tile([64, S], BF16, tag=f"ctx{p}", name=f"ctx{p}") for p in range(2 * HPC)]

        for p in range(2 * HPC):
            nc.vector.memset(v_aug[p][:, :, 64:65], 1.0)

        Ident = mybir.ActivationFunctionType.Identity
        Exp = mybir.ActivationFunctionType.Exp

        for _rep in range(repeat):
            # ---------------- phase 1: projections ----------------
            with ExitStack() as p1:
                wpool = p1.enter_context(tc.tile_pool(name="wpool", bufs=1))
                qst = p1.enter_context(tc.tile_pool(name="qst", bufs=2))
                kst = p1.enter_context(tc.tile_pool(name="kst", bufs=2))
                vst = p1.enter_context(tc.tile_pool(name="vst", bufs=2))
                psA = p1.enter_context(tc.tile_pool(name="psA", bufs=4, space="PSUM"))

                wq_sb = [[wpool.tile([128, NDK, 128], BF16, tag=f"wq{hl}{c}", name=f"wq{hl}{c}")
                          for c in range(3)] for hl in range(HPC)]
                wkey_sb = [wpool.tile([128, NDK, 128], BF16, tag=f"wk{hl}", name=f"wk{hl}")
                           for hl in range(HPC)]
                wch_sb = [wpool.tile([12, 128], BF16, tag=f"wch{hl}", name=f"wch{hl}") for hl in range(HPC)]
                wmb_sb = [wpool.tile([36, 128], BF16, tag=f"wmb{hl}", name=f"wmb{hl}") for hl in range(HPC)]
                wrhy_sb = [wpool.tile([4, 64], BF16, tag=f"wrhy{hl}", name=f"wrhy{hl}") for hl in range(HPC)]
                wv_sb = wpool.tile([128, NDK, 128], BF16, tag="wv")
                bq_sb = [[wpool.tile([128, 1], F32, tag=f"bq{hl}{c}", name=f"bq{hl}{c}")
                          for c in range(3)] for hl in range(HPC)]
                bk0_sb = [wpool.tile([128, 1], F32, tag=f"bk0{hl}", name=f"bk0{hl}") for hl in range(HPC)]
                bk1_sb = [wpool.tile([128, 1], F32, tag=f"bk1{hl}", name=f"bk1{hl}") for hl in range(HPC)]
                bk2_sb = [wpool.tile([64, 1], F32, tag=f"bk2{hl}", name=f"bk2{hl}") for hl in range(HPC)]
                fpitch = wpool.tile([12, S], BF16, tag="fpitch")
                fmb = wpool.tile([36, S], BF16, tag="fmb")
                frhy = wpool.tile([4, S], BF16, tag="frhy")

                pre = {}
                for nm, pool, srcap in (("qt", qst, qT_r), ("kt", kst, kT_r),
                                        ("vt", vst, vT_r)):
                    t = pool.tile([128, NDK, SCW], BF16, tag=nm, name=nm)
                    nc.sync.dma_start(out=t[:], in_=srcap[:, :, 0:SCW])
                    pre[nm] = t
                for hl in range(HPC):
                    for c in range(3):
                        nc.sync.dma_start(
                            out=wq_sb[hl][c][:],
                            in_=wq[hl, c].rearrange("(dk p) m -> p dk m", p=128))
                        nc.sync.dma_start(out=bq_sb[hl][c][:], in_=bq[hl, c])
                    nc.sync.dma_start(
                        out=wkey_sb[hl][:],
                        in_=wkey[hl].rearrange("(dk p) m -> p dk m", p=128))
                    nc.sync.dma_start(out=wch_sb[hl][:], in_=wch[hl])
                    nc.sync.dma_start(out=wmb_sb[hl][:], in_=wmb[hl])
                    nc.sync.dma_start(out=wrhy_sb[hl][:], in_=wrhy[hl])
                    nc.sync.dma_start(out=bk0_sb[hl][:], in_=bk0[hl])
                    nc.sync.dma_start(out=bk1_sb[hl][:], in_=bk1[hl])
                    nc.sync.dma_start(out=bk2_sb[hl][:], in_=bk2[hl])
                nc.sync.dma_start(out=wv_sb[:],
                                  in_=wv.rearrange("(dk p) m -> p dk m", p=128))
                nc.sync.dma_start(out=fpitch[:], in_=pitch2[:])
                nc.sync.dma_start(out=fmb[:], in_=melbass[:])
                nc.sync.dma_start(out=frhy[:], in_=rhy2[:])

                for j in range(NSC):
                    c0, c1 = j * SCW, (j + 1) * SCW
                    if j == 0:
                        qt, kt, vt = pre["qt"], pre["kt"], pre["vt"]
                    else:
                        qt = qst.tile([128, NDK, SCW], BF16, tag="qt")
                        kt = kst.tile([128, NDK, SCW], BF16, tag="kt")
                        vt = vst.tile([128, NDK, SCW], BF16, tag="vt")
                        nc.sync.dma_start(out=qt[:], in_=qT_r[:, :, c0:c1])
                        nc.sync.dma_start(out=kt[:], in_=kT_r[:, :, c0:c1])
                        nc.sync.dma_start(out=vt[:], in_=vT_r[:, :, c0:c1])

                    for hl in range(HPC):
                        for c in range(3):
                            ps = psA.tile([128, SCW], F32, tag="psq")
                            for dk in range(NDK):
                                nc.tensor.matmul(ps[:], wq_sb[hl][c][:, dk, :],
                                                 qt[:, dk, :],
                                                 start=(dk == 0), stop=(dk == NDK - 1))
                            nc.scalar.activation(qc_sb[hl][c][:, c0:c1], ps[:],
                                                 Ident, bias=bq_sb[hl][c][:], scale=1.0)
                    for hl in range(HPC):
                        ps = psA.tile([128, SCW], F32, tag="psq")
                        nc.tensor.matmul(ps[:], wch_sb[hl][:],
                                         fpitch[:, c0 % S:c0 % S + SCW],
                                         start=True, stop=False)
                        for dk in range(NDK):
                            nc.tensor.matmul(ps[:], wkey_sb[hl][:, dk, :],
                                             kt[:, dk, :],
                                             start=False, stop=(dk == NDK - 1))
                        nc.scalar.activation(kc0_sb[hl][:, c0:c1], ps[:],
                                             Ident, bias=bk0_sb[hl][:], scale=1.0)
                        if c1 <= S:
                            ps = psA.tile([128, SCW], F32, tag="psq")
                            nc.tensor.matmul(ps[:], wmb_sb[hl][:], fmb[:, c0:c1],
                                             start=True, stop=True)
                            nc.scalar.activation(kc1_sb[hl][:, c0:c1], ps[:],
                                                 Ident, bias=bk1_sb[hl][:], scale=1.0)
                            ps2 = psA.tile([64, SCW], F32, tag="psk2", bufs=2)
                            nc.tensor.matmul(ps2[:], wrhy_sb[hl][:], frhy[:, c0:c1],
                                             start=True, stop=True)
                            nc.scalar.activation(kc2_sb[hl][0:64, c0:c1], ps2[:],
                                                 Ident, bias=bk2_sb[hl][:], scale=1.0)
                    b = c0 // S
                    for m in range(SCW // 128):
                        psv = psA.tile([128, 128], F32, tag="psv", bufs=2)
                        for dk in range(NDK):
                            nc.tensor.matmul(psv[:], vt[:, dk, m * 128:(m + 1) * 128],
                                             wv_sb[:, dk, :],
                                             start=(dk == 0), stop=(dk == NDK - 1))
                        kti = (c0 % S) // 128 + m
                        for hl in range(HPC):
                            nc.vector.tensor_copy(
                                v_aug[hl * 2 + b][:, kti, 0:64],
                                psv[:, hl * 64:(hl + 1) * 64])

            # ------- phase 2: attention per (hl, b) pair + inline out-proj -------
            with ExitStack() as p2:
                psS = p2.enter_context(tc.tile_pool(name="psS", bufs=5, space="PSUM"))
                psC = p2.enter_context(tc.tile_pool(name="psC", bufs=3, space="PSUM"))
                esp = p2.enter_context(tc.tile_pool(name="esp", bufs=2 * NKT + 2))
                cxs = p2.enter_context(tc.tile_pool(name="cxs", bufs=3))
                msc = p2.enter_context(tc.tile_pool(name="msc", bufs=2))
                att = p2.enter_context(tc.tile_pool(name="att", bufs=4))
                st3 = p2.enter_context(tc.tile_pool(name="st3", bufs=3))
                dsc = p2.enter_context(tc.tile_pool(name="dsc", bufs=4, space="DRAM"))
                w3 = p2.enter_context(tc.tile_pool(name="w3", bufs=1))
                wout_sb = [w3.tile([64, H], BF16, tag=f"wo{hl}", name=f"wo{hl}")
                           for hl in range(HPC)]
                for hl in range(HPC):
                    nc.sync.dma_start(out=wout_sb[hl][:], in_=wout[hl])
                    # rows 64-127 <- rows 0-63 shifted left 128 cols (next k-tile)
                    nc.sync.dma_start(out=kc2_sb[hl][64:128, 0:S - 128],
                                      in_=kc2_sb[hl][0:64, 128:S])

                for hl in range(HPC):
                    for b in range(B):
                        p = hl * 2 + b
                        bc = b * S
                        for qc in range(NQC):
                            q0 = qc * QCH
                            ctx_ps = psC.tile([65, QCH], F32, tag="ctx")
                            exp_tiles = []
                            for kt2 in range(0, NKT, 2):
                                spss = [psS.tile([128, QCH], F32, tag="sc", name="sps")
                                        for _ in range(2)]
                                for i in range(2):
                                    k0 = (kt2 + i) * 128
                                    for n0 in range(0, QCH, 512):
                                        n1 = min(n0 + 512, QCH)
                                        nc.tensor.matmul(
                                            spss[i][:, n0:n1],
                                            kc0_sb[hl][:, bc + k0:bc + k0 + 128],
                                            qc_sb[hl][0][:, bc + q0 + n0:bc + q0 + n1],
                                            start=True, stop=False)
                                        nc.tensor.matmul(
                                            spss[i][:, n0:n1], kc1_sb[hl][:, k0:k0 + 128],
                                            qc_sb[hl][1][:, bc + q0 + n0:bc + q0 + n1],
                                            start=False, stop=False)
                                # c2 (rhythm): both k-tiles concurrently via row tiling
                                k0 = kt2 * 128
                                for n0 in range(0, QCH, 512):
                                    n1 = min(n0 + 512, QCH)
                                    nc.tensor.matmul(
                                        spss[0][:, n0:n1], kc2_sb[hl][0:64, k0:k0 + 128],
                                        qc_sb[hl][2][0:64, bc + q0 + n0:bc + q0 + n1],
                                        start=False, stop=True, tile_position=(0, 0))
                                    nc.tensor.matmul(
                                        spss[1][:, n0:n1], kc2_sb[hl][64:128, k0:k0 + 128],
                                        qc_sb[hl][2][64:128, bc + q0 + n0:bc + q0 + n1],
                                        start=False, stop=True, tile_position=(64, 0))
                                for i in range(2):
                                    kti = kt2 + i
                                    et = esp.tile([128, QCH], BF16, tag="et")
                                    exp_tiles.append(et)
                                    nc.scalar.activation(et[:], spss[i][:], Exp,
                                                         bias=0.0, scale=SCALE)
                                    for n0 in range(0, QCH, 512):
                                        n1 = min(n0 + 512, QCH)
                                        nc.tensor.matmul(ctx_ps[:, n0:n1],
                                                         v_aug[p][:, kti, :],
                                                         et[:, n0:n1],
                                                         start=(kti == 0),
                                                         stop=(kti == NKT - 1))
                            cx = cxs.tile([65, QCH], F32, tag="cx")
                            nc.scalar.activation(cx[:], ctx_ps[:], Ident,
                                                 bias=0.0, scale=1.0)
                            d_f32 = dsc.tile([1, QCH], F32, tag="d32")
                            nc.gpsimd.dma_start(out=d_f32[:], in_=cx[64:65, :])
                            rsp = msc.tile([128, QCH // 128], F32, tag="rsp")
                            nc.gpsimd.dma_start(
                                out=rsp[:],
                                in_=d_f32.rearrange("o (p m) -> (o p) m", p=128))
                            nc.vector.reciprocal(rsp[:], rsp[:])
                            r16 = msc.tile([128, QCH // 128], BF16, tag="r16")
                            nc.vector.tensor_copy(r16[:], rsp[:])
                            d_bf = dsc.tile([1, QCH], BF16, tag="dbf")
                            nc.gpsimd.dma_start(
                                out=d_bf.rearrange("o (p m) -> (o p) m", p=128),
                                in_=r16[:])
                            rbc = msc.tile([128, QCH], BF16, tag="rbc")
                            nc.gpsimd.dma_start(
                                out=rbc[:], in_=d_bf.partition_broadcast(128))
                            nc.vector.tensor_tensor(
                                out=ctx_sb[p][:, q0:q0 + QCH], in0=cx[0:64, :],
                                in1=rbc[0:64, :], op=mybir.AluOpType.mult)
                            for kti in range(NKT):
                                at = att.tile([128, QCH], BF16, tag="at")
                                nc.vector.tensor_tensor(
                                    out=at[:], in0=exp_tiles[kti][:], in1=rbc[:],
                                    op=mybir.AluOpType.mult)
                                nc.scalar.dma_start(
                                    out=attnT[p, kti * 128:(kti + 1) * 128,
                                              q0:q0 + QCH],
                                    in_=at[:])
                        if hl == HPC - 1:
                            for m in range(S // 128):
                                m0 = m * 128
                                ot = st3.tile([128, H], BF16, tag="ot")
                                for n in range(H // 512):
                                    n0, n1 = n * 512, (n + 1) * 512
                                    pso = psC.tile([128, 512], F32, tag="ctx")
                                    for h2 in range(HPC):
                                        nc.tensor.matmul(
                                            pso[:], ctx_sb[h2 * 2 + b][:, m0:m0 + 128],
                                            wout_sb[h2][:, n0:n1],
                                            start=(h2 == 0), stop=(h2 == HPC - 1))
                                    nc.vector.tensor_copy(ot[:, n0:n1], pso[:])
                                nc.gpsimd.dma_start(
                                    out=outp[b * S + m0:b * S + m0 + 128, :],
                                    in_=ot[:])
    nc.compile()
    return nc


def _get_program(S, repeat=1):
    key = (S, repeat)
    if key not in _programs:
        nc = bacc.Bacc("TRN2", debug=False)
        _programs[key] = _emit(nc, S, repeat)
    return _programs[key]


def _prep(inputs):
    """Host-side shard prep. Returns (in_maps, aux) for the 8 cores."""
    q = np.asarray(inputs["query"], np.float32)
    k = np.asarray(inputs["key"], np.float32)
    v = np.asarray(inputs["value"], np.float32)
    S = q.shape[1]
    BS = B * S
    params = {name: {"w": np.asarray(pp["w"], np.float32),
                     "b": np.asarray(pp["b"], np.float32)}
              for name, pp in inputs["params"].items()}

    qTn = np.ascontiguousarray(q.reshape(BS, H).T).astype(BF)
    kTn = np.ascontiguousarray(k.reshape(BS, H).T).astype(BF)
    vTn = np.ascontiguousarray(v.reshape(BS, H).T).astype(BF)
    pitch2 = np.asarray(inputs["pitch_classes"], np.float32).T.astype(BF)
    melbass = np.concatenate(
        [np.asarray(inputs["melody"], np.float32).T,
         np.asarray(inputs["bass"], np.float32).T], axis=0).astype(BF)
    rhy2 = np.asarray(inputs["rhythm"], np.float32).T.astype(BF)

    shared = {"qT": qTn, "kT": kTn, "vT": vTn,
              "pitch2": pitch2, "melbass": melbass, "rhy2": rhy2}

    in_maps = []
    for c in range(NCORES):
        m = dict(shared)
        wq = np.zeros((HPC, 3, H, 128), np.float32)
        bq = np.zeros((HPC, 3, 128, 1), np.float32)
        wkey = np.zeros((HPC, H, 128), np.float32)
        wch = np.zeros((HPC, 12, 128), np.float32)
        wmb = np.zeros((HPC, 36, 128), np.float32)
        wrhy = np.zeros((HPC, 4, 64), np.float32)
        bk0 = np.zeros((HPC, 128, 1), np.float32)
        bk1 = np.zeros((HPC, 128, 1), np.float32)
        bk2 = np.zeros((HPC, 64, 1), np.float32)
        wout = np.zeros((HPC, 64, H), np.float32)
        for hl in range(HPC):
            h = c * HPC + hl
            sl = slice(h * HD, (h + 1) * HD)
            # q_cat chunks: c0=[qp|qh] c1=[qm|qv] c2=[qr|0]
            wq[hl, 0, :, 0:64] = params["pitch_query"]["w"][:, sl]
            wq[hl, 0, :, 64:128] = params["harmony_query"]["w"][:, sl]
            wq[hl, 1, :, 0:64] = params["melody_query"]["w"][:, sl]
            wq[hl, 1, :, 64:128] = params["voice_query"]["w"][:, sl]
            wq[hl, 2, :, 0:64] = params["rhythm_query"]["w"][:, sl]
            wq[hl, 2, :, 64:128] = params["rhythm_query"]["w"][:, sl]
            bq[hl, 0, 0:64, 0] = params["pitch_query"]["b"][sl]
            bq[hl, 0, 64:128, 0] = params["harmony_query"]["b"][sl]
            bq[hl, 1, 0:64, 0] = params["melody_query"]["b"][sl]
            bq[hl, 1, 64:128, 0] = params["voice_query"]["b"][sl]
            bq[hl, 2, 0:64, 0] = params["rhythm_query"]["b"][sl]
            bq[hl, 2, 64:128, 0] = params["rhythm_query"]["b"][sl]
            # k_cat chunks: c0=[k|chord] c1=[mel|bass] c2=[rhy]
            wkey[hl, :, 0:64] = params["key"]["w"][:, sl]
            wch[hl, :, 64:128] = params["chord_proj"]["w"][:, sl]
            wmb[hl, 0:24, 0:64] = params["melody_proj"]["w"][:, sl]
            wmb[hl, 24:36, 64:128] = params["bass_proj"]["w"][:, sl]
            wrhy[hl] = params["rhythm_proj"]["w"][:, sl]
            bk0[hl, 0:64, 0] = params["key"]["b"][sl]
            bk0[hl, 64:128, 0] = params["chord_proj"]["b"][sl]
            bk1[hl, 0:64, 0] = params["melody_proj"]["b"][sl]
            bk1[hl, 64:128, 0] = params["bass_proj"]["b"][sl]
            bk2[hl, 0:64, 0] = params["rhythm_proj"]["b"][sl]
            wout[hl] = params["out"]["w"][sl, :]
        wv_c = params["value"]["w"][:, c * HPC * HD:(c + 1) * HPC * HD]
        m.update(wq=wq.astype(BF), bq=bq, wkey=wkey.astype(BF),
                 wch=wch.astype(BF), wmb=wmb.astype(BF), wrhy=wrhy.astype(BF),
                 bk0=bk0, bk1=bk1, bk2=bk2, wv=wv_c.astype(BF),
                 wout=wout.astype(BF))
        in_maps.append(m)

    # host-side constant bias row: out bias + value-bias routed through out proj
    bias_total = params["out"]["b"] + params["value"]["b"] @ params["out"]["w"]
    return in_maps, bias_total, S


def _assemble(results, bias_total, S):
    out = np.zeros((B * S, H), np.float32)
    attn = np.empty((B, NH, S, S), np.float32)
    for c in range(NCORES):
        out += results[c]["outp"].astype(np.float32)
        at = results[c]["attnT"]
        for hl in range(HPC):
            for b in range(B):
                attn[b, c * HPC + hl] = at[hl * 2 + b].astype(np.float32)
    out += bias_total[None, :]
    out = out.reshape(B, S, H)
    return out, attn.transpose(0, 1, 3, 2)


def kernel(_repeat=1, _sim=False, **inputs):
    in_maps, bias_total, S = _prep(inputs)
    if _sim:
        from concourse.bass_interp import CoreSim
        nc = _get_program(S, _repeat)
        results = []
        for c in range(NCORES):
            sim = CoreSim(nc)
            for k2, v2 in in_maps[c].items():
                sim.tensor(k2)[:] = v2
            sim.simulate()
            results.append({"attnT": sim.tensor("attnT").copy(),
                            "outp": sim.tensor("outp").copy()})
    else:
        nc = _get_program(S, _repeat)
        res = bass_utils.run_bass_kernel_spmd(nc, in_maps,
                                              core_ids=list(range(NCORES)))
        results = res.results
    return _assemble(results, bias_total, S)


# revision 20
# speedup vs baseline: 2.0502x; 2.0502x over previous
"""ChordAwareAttention Trainium2 kernel.

Sharding: 16 heads across 8 cores (2 heads/core), both batches on every core
(4 (b,h) pairs per core). Inputs are host-transposed to d-major [H, B*S] so all
projections run on-device; scores are computed transposed ST[k,q] so the
softmax denominator falls out of the ctx matmul (ones column in v_aug) and ctx
needs no on-chip transpose. attn is written to DRAM as [k,q] bf16 per pair and
returned to the caller as a zero-copy transposed fp32 view.
"""
import sys

try:
    import concourse.bass as bass  # noqa: F401
except ImportError:
    sys.path.insert(0, "/opt/trn_rl_repo")

import numpy as np
import ml_dtypes

import concourse.bass as bass
import concourse.mybir as mybir
import concourse.tile as tile
from concourse import bacc, bass_utils
from contextlib import ExitStack

F32 = mybir.dt.float32
BF16 = mybir.dt.bfloat16
BF = ml_dtypes.bfloat16

B, H, NH, HD = 2, 1024, 16, 64
NCORES = 8
HPC = NH // NCORES          # heads per core
SCALE = (HD ** -0.5) / 5.0  # applied inside exp

_programs = {}


def _emit(nc, S, repeat):
    BS = B * S
    SCW = min(512, S)        # phase-1 seq chunk width
    NSC = BS // SCW
    NKT = S // 128           # 128-row k tiles per pair
    QCH = min(512, S)        # phase-2 q chunk width
    NQC = S // QCH           # q chunks per pair
    NDK = H // 128           # 8 din chunks

    qT = nc.dram_tensor("qT", [H, BS], BF16, kind="ExternalInput").ap()
    kT = nc.dram_tensor("kT", [H, BS], BF16, kind="ExternalInput").ap()
    vT = nc.dram_tensor("vT", [H, BS], BF16, kind="ExternalInput").ap()
    pitch2 = nc.dram_tensor("pitch2", [12, S], BF16, kind="ExternalInput").ap()
    melbass = nc.dram_tensor("melbass", [36, S], BF16, kind="ExternalInput").ap()
    rhy2 = nc.dram_tensor("rhy2", [4, S], BF16, kind="ExternalInput").ap()

    wq = nc.dram_tensor("wq", [HPC, 3, H, 128], BF16, kind="ExternalInput").ap()
    bq = nc.dram_tensor("bq", [HPC, 3, 128, 1], F32, kind="ExternalInput").ap()
    wkey = nc.dram_tensor("wkey", [HPC, H, 128], BF16, kind="ExternalInput").ap()
    wch = nc.dram_tensor("wch", [HPC, 12, 128], BF16, kind="ExternalInput").ap()
    wmb = nc.dram_tensor("wmb", [HPC, 36, 128], BF16, kind="ExternalInput").ap()
    wrhy = nc.dram_tensor("wrhy", [HPC, 4, 64], BF16, kind="ExternalInput").ap()
    bk0 = nc.dram_tensor("bk0", [HPC, 128, 1], F32, kind="ExternalInput").ap()
    bk1 = nc.dram_tensor("bk1", [HPC, 128, 1], F32, kind="ExternalInput").ap()
    bk2 = nc.dram_tensor("bk2", [HPC, 64, 1], F32, kind="ExternalInput").ap()
    wv = nc.dram_tensor("wv", [H, 128], BF16, kind="ExternalInput").ap()
    wout = nc.dram_tensor("wout", [HPC, 64, H], BF16, kind="ExternalInput").ap()

    attnT = nc.dram_tensor("attnT", [2 * HPC, S, S], BF16, kind="ExternalOutput").ap()
    outp = nc.dram_tensor("outp", [BS, H], BF16, kind="ExternalOutput").ap()

    qT_r = qT.rearrange("(dk p) s -> p dk s", p=128)
    kT_r = kT.rearrange("(dk p) s -> p dk s", p=128)
    vT_r = vT.rearrange("(dk p) s -> p dk s", p=128)

    with tile.TileContext(nc) as tc, ExitStack() as top:
        singles = top.enter_context(tc.tile_pool(name="singles", bufs=1))
        qc_sb = [[singles.tile([128, BS], BF16, tag=f"qc{hl}{c}", name=f"qc{hl}{c}")
                  for c in range(3)] for hl in range(HPC)]
        kc0_sb = [singles.tile([128, BS], BF16, tag=f"kc0{hl}", name=f"kc0{hl}") for hl in range(HPC)]
        kc1_sb = [singles.tile([128, S], BF16, tag=f"kc1{hl}", name=f"kc1{hl}") for hl in range(HPC)]
        kc2_sb = [singles.tile([128, S], BF16, tag=f"kc2{hl}", name=f"kc2{hl}") for hl in range(HPC)]
        v_aug = [singles.tile([128, NKT, 65], BF16, tag=f"vaug{p}", name=f"vaug{p}")
                 for p in range(2 * HPC)]
        ctx_sb = [singles.tile([64, S], BF16, tag=f"ctx{p}", name=f"ctx{p}") for p in range(2 * HPC)]

        for p in range(2 * HPC):
            nc.vector.memset(v_aug[p][:, :, 64:65], 1.0)

        Ident = mybir.ActivationFunctionType.Identity
        Exp = mybir.ActivationFunctionType.Exp

        for _rep in range(repeat):
            # ---------------- phase 1: projections ----------------
            with ExitStack() as p1:
                wpool = p1.enter_context(tc.tile_pool(name="wpool", bufs=1))
                qst = p1.enter_context(tc.tile_pool(name="qst", bufs=2))
                kst = p1.enter_context(tc.tile_pool(name="kst", bufs=2))
                vst = p1.enter_context(tc.tile_pool(name="vst", bufs=2))
                psA = p1.enter_context(tc.tile_pool(name="psA", bufs=5, space="PSUM"))

                wq_sb = [[wpool.tile([128, NDK, 128], BF16, tag=f"wq{hl}{c}", name=f"wq{hl}{c}")
                          for c in range(3)] for hl in range(HPC)]
                wkey_sb = [wpool.tile([128, NDK, 128], BF16, tag=f"wk{hl}", name=f"wk{hl}")
                           for hl in range(HPC)]
                wch_sb = [wpool.tile([12, 128], BF16, tag=f"wch{hl}", name=f"wch{hl}") for hl in range(HPC)]
                wmb_sb = [wpool.tile([36, 128], BF16, tag=f"wmb{hl}", name=f"wmb{hl}") for hl in range(HPC)]
                wrhy_sb = [wpool.tile([4, 64], BF16, tag=f"wrhy{hl}", name=f"wrhy{hl}") for hl in range(HPC)]
                wv_sb = wpool.tile([128, NDK, 128], BF16, tag="wv")
                bq_sb = [[wpool.tile([128, 1], F32, tag=f"bq{hl}{c}", name=f"bq{hl}{c}")
                          for c in range(3)] for hl in range(HPC)]
                bk0_sb = [wpool.tile([128, 1], F32, tag=f"bk0{hl}", name=f"bk0{hl}") for hl in range(HPC)]
                bk1_sb = [wpool.tile([128, 1], F32, tag=f"bk1{hl}", name=f"bk1{hl}") for hl in range(HPC)]
                bk2_sb = [wpool.tile([64, 1], F32, tag=f"bk2{hl}", name=f"bk2{hl}") for hl in range(HPC)]
                fpitch = wpool.tile([12, S], BF16, tag="fpitch")
                fmb = wpool.tile([36, S], BF16, tag="fmb")
                frhy = wpool.tile([4, S], BF16, tag="frhy")

                pre = {}
                for nm, pool, srcap in (("qt", qst, qT_r), ("kt", kst, kT_r),
                                        ("vt", vst, vT_r)):
                    t = pool.tile([128, NDK, SCW], BF16, tag=nm, name=nm)
                    nc.sync.dma_start(out=t[:], in_=srcap[:, :, 0:SCW])
                    pre[nm] = t
                for hl in range(HPC):
                    for c in range(3):
                        nc.sync.dma_start(
                            out=wq_sb[hl][c][:],
                            in_=wq[hl, c].rearrange("(dk p) m -> p dk m", p=128))
                        nc.sync.dma_start(out=bq_sb[hl][c][:], in_=bq[hl, c])
                    nc.sync.dma_start(
                        out=wkey_sb[hl][:],
                        in_=wkey[hl].rearrange("(dk p) m -> p dk m", p=128))
                    nc.sync.dma_start(out=wch_sb[hl][:], in_=wch[hl])
                    nc.sync.dma_start(out=wmb_sb[hl][:], in_=wmb[hl])
                    nc.sync.dma_start(out=wrhy_sb[hl][:], in_=wrhy[hl])
                    nc.sync.dma_start(out=bk0_sb[hl][:], in_=bk0[hl])
                    nc.sync.dma_start(out=bk1_sb[hl][:], in_=bk1[hl])
                    nc.sync.dma_start(out=bk2_sb[hl][:], in_=bk2[hl])
                nc.sync.dma_start(out=wv_sb[:],
                                  in_=wv.rearrange("(dk p) m -> p dk m", p=128))
                nc.sync.dma_start(out=fpitch[:], in_=pitch2[:])
                nc.sync.dma_start(out=fmb[:], in_=melbass[:])
                nc.sync.dma_start(out=frhy[:], in_=rhy2[:])

                for j in range(NSC):
                    c0, c1 = j * SCW, (j + 1) * SCW
                    if j == 0:
                        qt, kt, vt = pre["qt"], pre["kt"], pre["vt"]
                    else:
                        qt = qst.tile([128, NDK, SCW], BF16, tag="qt")
                        kt = kst.tile([128, NDK, SCW], BF16, tag="kt")
                        vt = vst.tile([128, NDK, SCW], BF16, tag="vt")
                        nc.sync.dma_start(out=qt[:], in_=qT_r[:, :, c0:c1])
                        nc.sync.dma_start(out=kt[:], in_=kT_r[:, :, c0:c1])
                        nc.sync.dma_start(out=vt[:], in_=vT_r[:, :, c0:c1])

                    for hl in range(HPC):
                        for c in range(3):
                            ps = psA.tile([128, SCW], F32, tag="psq")
                            for dk in range(NDK):
                                nc.tensor.matmul(ps[:], wq_sb[hl][c][:, dk, :],
                                                 qt[:, dk, :],
                                                 start=(dk == 0), stop=(dk == NDK - 1))
                            nc.scalar.activation(qc_sb[hl][c][:, c0:c1], ps[:],
                                                 Ident, bias=bq_sb[hl][c][:], scale=1.0)
                    for hl in range(HPC):
                        ps = psA.tile([128, SCW], F32, tag="psq")
                        nc.tensor.matmul(ps[:], wch_sb[hl][:],
                                         fpitch[:, c0 % S:c0 % S + SCW],
                                         start=True, stop=False)
                        for dk in range(NDK):
                            nc.tensor.matmul(ps[:], wkey_sb[hl][:, dk, :],
                                             kt[:, dk, :],
                                             start=False, stop=(dk == NDK - 1))
                        nc.scalar.activation(kc0_sb[hl][:, c0:c1], ps[:],
                                             Ident, bias=bk0_sb[hl][:], scale=1.0)
                        if c1 <= S:
                            ps = psA.tile([128, SCW], F32, tag="psq")
                            nc.tensor.matmul(ps[:], wmb_sb[hl][:], fmb[:, c0:c1],
                                             start=True, stop=True)
                            nc.scalar.activation(kc1_sb[hl][:, c0:c1], ps[:],
                                                 Ident, bias=bk1_sb[hl][:], scale=1.0)
                            ps2 = psA.tile([64, SCW], F32, tag="psk2", bufs=1)
                            nc.tensor.matmul(ps2[:], wrhy_sb[hl][:], frhy[:, c0:c1],
                                             start=True, stop=True)
                            nc.scalar.activation(kc2_sb[hl][0:64, c0:c1], ps2[:],
                                                 Ident, bias=bk2_sb[hl][:], scale=1.0)
                    b = c0 // S
                    for m in range(SCW // 128):
                        psv = psA.tile([128, 128], F32, tag="psv", bufs=2)
                        for dk in range(NDK):
                            nc.tensor.matmul(psv[:], vt[:, dk, m * 128:(m + 1) * 128],
                                             wv_sb[:, dk, :],
                                             start=(dk == 0), stop=(dk == NDK - 1))
                        kti = (c0 % S) // 128 + m
                        for hl in range(HPC):
                            nc.vector.tensor_copy(
                                v_aug[hl * 2 + b][:, kti, 0:64],
                                psv[:, hl * 64:(hl + 1) * 64])

            # ------- phase 2: attention per (hl, b) pair + inline out-proj -------
            with ExitStack() as p2:
                psS = p2.enter_context(tc.tile_pool(name="psS", bufs=5, space="PSUM"))
                psC = p2.enter_context(tc.tile_pool(name="psC", bufs=3, space="PSUM"))
                esp = p2.enter_context(tc.tile_pool(name="esp", bufs=2 * NKT + 2))
                cxs = p2.enter_context(tc.tile_pool(name="cxs", bufs=3))
                msc = p2.enter_context(tc.tile_pool(name="msc", bufs=2))
                att = p2.enter_context(tc.tile_pool(name="att", bufs=4))
                st3 = p2.enter_context(tc.tile_pool(name="st3", bufs=3))
                dsc = p2.enter_context(tc.tile_pool(name="dsc", bufs=4, space="DRAM"))
                w3 = p2.enter_context(tc.tile_pool(name="w3", bufs=1))
                wout_sb = [w3.tile([64, H], BF16, tag=f"wo{hl}", name=f"wo{hl}")
                           for hl in range(HPC)]
                for hl in range(HPC):
                    nc.sync.dma_start(out=wout_sb[hl][:], in_=wout[hl])
                    # rows 64-127 <- rows 0-63 shifted left 128 cols (next k-tile)
                    nc.sync.dma_start(out=kc2_sb[hl][64:128, 0:S - 128],
                                      in_=kc2_sb[hl][0:64, 128:S])

                for hl in range(HPC):
                    for b in range(B):
                        p = hl * 2 + b
                        bc = b * S
                        for qc in range(NQC):
                            q0 = qc * QCH
                            ctx_ps = psC.tile([65, QCH], F32, tag="ctx")
                            exp_tiles = []
                            for kt2 in range(0, NKT, 2):
                                spss = [psS.tile([128, QCH], F32, tag="sc", name="sps")
                                        for _ in range(2)]
                                for i in range(2):
                                    k0 = (kt2 + i) * 128
                                    for n0 in range(0, QCH, 512):
                                        n1 = min(n0 + 512, QCH)
                                        nc.tensor.matmul(
                                            spss[i][:, n0:n1],
                                            kc0_sb[hl][:, bc + k0:bc + k0 + 128],
                                            qc_sb[hl][0][:, bc + q0 + n0:bc + q0 + n1],
                                            start=True, stop=False)
                                        nc.tensor.matmul(
                                            spss[i][:, n0:n1], kc1_sb[hl][:, k0:k0 + 128],
                                            qc_sb[hl][1][:, bc + q0 + n0:bc + q0 + n1],
                                            start=False, stop=False)
                                # c2 (rhythm): both k-tiles concurrently via row tiling
                                k0 = kt2 * 128
                                for n0 in range(0, QCH, 512):
                                    n1 = min(n0 + 512, QCH)
                                    nc.tensor.matmul(
                                        spss[0][:, n0:n1], kc2_sb[hl][0:64, k0:k0 + 128],
                                        qc_sb[hl][2][0:64, bc + q0 + n0:bc + q0 + n1],
                                        start=False, stop=True, tile_position=(0, 0))
                                    nc.tensor.matmul(
                                        spss[1][:, n0:n1], kc2_sb[hl][64:128, k0:k0 + 128],
                                        qc_sb[hl][2][64:128, bc + q0 + n0:bc + q0 + n1],
                                        start=False, stop=True, tile_position=(64, 0))
                                for i in range(2):
                                    kti = kt2 + i
                                    et = esp.tile([128, QCH], BF16, tag="et")
                                    exp_tiles.append(et)
                                    nc.scalar.activation(et[:], spss[i][:], Exp,
                                                         bias=0.0, scale=SCALE)
                                    for n0 in range(0, QCH, 512):
                                        n1 = min(n0 + 512, QCH)
                                        nc.tensor.matmul(ctx_ps[:, n0:n1],
                                                         v_aug[p][:, kti, :],
                                                         et[:, n0:n1],
                                                         start=(kti == 0),
                                                         stop=(kti == NKT - 1))
                            cx = cxs.tile([65, QCH], F32, tag="cx")
                            nc.scalar.activation(cx[:], ctx_ps[:], Ident,
                                                 bias=0.0, scale=1.0)
                            d_f32 = dsc.tile([1, QCH], F32, tag="d32")
                            nc.gpsimd.dma_start(out=d_f32[:], in_=cx[64:65, :])
                            rsp = msc.tile([128, QCH // 128], F32, tag="rsp")
                            nc.gpsimd.dma_start(
                                out=rsp[:],
                                in_=d_f32.rearrange("o (p m) -> (o p) m", p=128))
                            nc.vector.reciprocal(rsp[:], rsp[:])
                            r16 = msc.tile([128, QCH // 128], BF16, tag="r16")
                            nc.vector.tensor_copy(r16[:], rsp[:])
                            d_bf = dsc.tile([1, QCH], BF16, tag="dbf")
                            nc.gpsimd.dma_start(
                                out=d_bf.rearrange("o (p m) -> (o p) m", p=128),
                                in_=r16[:])
                            rbc = msc.tile([128, QCH], BF16, tag="rbc")
                            nc.gpsimd.dma_start(
                                out=rbc[:], in_=d_bf.partition_broadcast(128))
                            nc.vector.tensor_tensor(
                                out=ctx_sb[p][:, q0:q0 + QCH], in0=cx[0:64, :],
                                in1=rbc[0:64, :], op=mybir.AluOpType.mult)
                            for kti in range(NKT):
                                at = att.tile([128, QCH], BF16, tag="at")
                                nc.vector.tensor_tensor(
                                    out=at[:], in0=exp_tiles[kti][:], in1=rbc[:],
                                    op=mybir.AluOpType.mult)
                                nc.scalar.dma_start(
                                    out=attnT[p, kti * 128:(kti + 1) * 128,
                                              q0:q0 + QCH],
                                    in_=at[:])
                        if hl == HPC - 1:
                            for m in range(S // 128):
                                m0 = m * 128
                                ot = st3.tile([128, H], BF16, tag="ot")
                                for n in range(H // 512):
                                    n0, n1 = n * 512, (n + 1) * 512
                                    pso = psC.tile([128, 512], F32, tag="ctx")
                                    for h2 in range(HPC):
                                        nc.tensor.matmul(
                                            pso[:], ctx_sb[h2 * 2 + b][:, m0:m0 + 128],
                                            wout_sb[h2][:, n0:n1],
                                            start=(h2 == 0), stop=(h2 == HPC - 1))
                                    nc.vector.tensor_copy(ot[:, n0:n1], pso[:])
                                nc.gpsimd.dma_start(
                                    out=outp[b * S + m0:b * S + m0 + 128, :],
                                    in_=ot[:])
    nc.compile()
    return nc


def _get_program(S, repeat=1):
    key = (S, repeat)
    if key not in _programs:
        nc = bacc.Bacc("TRN2", debug=False)
        _programs[key] = _emit(nc, S, repeat)
    return _programs[key]


def _prep(inputs):
    """Host-side shard prep. Returns (in_maps, aux) for the 8 cores."""
    q = np.asarray(inputs["query"], np.float32)
    k = np.asarray(inputs["key"], np.float32)
    v = np.asarray(inputs["value"], np.float32)
    S = q.shape[1]
    BS = B * S
    params = {name: {"w": np.asarray(pp["w"], np.float32),
                     "b": np.asarray(pp["b"], np.float32)}
              for name, pp in inputs["params"].items()}

    qTn = np.ascontiguousarray(q.reshape(BS, H).T).astype(BF)
    kTn = np.ascontiguousarray(k.reshape(BS, H).T).astype(BF)
    vTn = np.ascontiguousarray(v.reshape(BS, H).T).astype(BF)
    pitch2 = np.asarray(inputs["pitch_classes"], np.float32).T.astype(BF)
    melbass = np.concatenate(
        [np.asarray(inputs["melody"], np.float32).T,
         np.asarray(inputs["bass"], np.float32).T], axis=0).astype(BF)
    rhy2 = np.asarray(inputs["rhythm"], np.float32).T.astype(BF)

    shared = {"qT": qTn, "kT": kTn, "vT": vTn,
              "pitch2": pitch2, "melbass": melbass, "rhy2": rhy2}

    in_maps = []
    for c in range(NCORES):
        m = dict(shared)
        wq = np.zeros((HPC, 3, H, 128), np.float32)
        bq = np.zeros((HPC, 3, 128, 1), np.float32)
        wkey = np.zeros((HPC, H, 128), np.float32)
        wch = np.zeros((HPC, 12, 128), np.float32)
        wmb = np.zeros((HPC, 36, 128), np.float32)
        wrhy = np.zeros((HPC, 4, 64), np.float32)
        bk0 = np.zeros((HPC, 128, 1), np.float32)
        bk1 = np.zeros((HPC, 128, 1), np.float32)
        bk2 = np.zeros((HPC, 64, 1), np.float32)
        wout = np.zeros((HPC, 64, H), np.float32)
        for hl in range(HPC):
            h = c * HPC + hl
            sl = slice(h * HD, (h + 1) * HD)
            # q_cat chunks: c0=[qp|qh] c1=[qm|qv] c2=[qr|0]
            wq[hl, 0, :, 0:64] = params["pitch_query"]["w"][:, sl]
            wq[hl, 0, :, 64:128] = params["harmony_query"]["w"][:, sl]
            wq[hl, 1, :, 0:64] = params["melody_query"]["w"][:, sl]
            wq[hl, 1, :, 64:128] = params["voice_query"]["w"][:, sl]
            wq[hl, 2, :, 0:64] = params["rhythm_query"]["w"][:, sl]
            wq[hl, 2, :, 64:128] = params["rhythm_query"]["w"][:, sl]
            bq[hl, 0, 0:64, 0] = params["pitch_query"]["b"][sl]
            bq[hl, 0, 64:128, 0] = params["harmony_query"]["b"][sl]
            bq[hl, 1, 0:64, 0] = params["melody_query"]["b"][sl]
            bq[hl, 1, 64:128, 0] = params["voice_query"]["b"][sl]
            bq[hl, 2, 0:64, 0] = params["rhythm_query"]["b"][sl]
            bq[hl, 2, 64:128, 0] = params["rhythm_query"]["b"][sl]
            # k_cat chunks: c0=[k|chord] c1=[mel|bass] c2=[rhy]
            wkey[hl, :, 0:64] = params["key"]["w"][:, sl]
            wch[hl, :, 64:128] = params["chord_proj"]["w"][:, sl]
            wmb[hl, 0:24, 0:64] = params["melody_proj"]["w"][:, sl]
            wmb[hl, 24:36, 64:128] = params["bass_proj"]["w"][:, sl]
            wrhy[hl] = params["rhythm_proj"]["w"][:, sl]
            bk0[hl, 0:64, 0] = params["key"]["b"][sl]
            bk0[hl, 64:128, 0] = params["chord_proj"]["b"][sl]
            bk1[hl, 0:64, 0] = params["melody_proj"]["b"][sl]
            bk1[hl, 64:128, 0] = params["bass_proj"]["b"][sl]
            bk2[hl, 0:64, 0] = params["rhythm_proj"]["b"][sl]
            wout[hl] = params["out"]["w"][sl, :]
        wv_c = params["value"]["w"][:, c * HPC * HD:(c + 1) * HPC * HD]
        m.update(wq=wq.astype(BF), bq=bq, wkey=wkey.astype(BF),
                 wch=wch.astype(BF), wmb=wmb.astype(BF), wrhy=wrhy.astype(BF),
                 bk0=bk0, bk1=bk1, bk2=bk2, wv=wv_c.astype(BF),
                 wout=wout.astype(BF))
        in_maps.append(m)

    # host-side constant bias row: out bias + value-bias routed through out proj
    bias_total = params["out"]["b"] + params["value"]["b"] @ params["out"]["w"]
    return in_maps, bias_total, S


def _assemble(results, bias_total, S):
    out = np.zeros((B * S, H), np.float32)
    attn = np.empty((B, NH, S, S), np.float32)
    for c in range(NCORES):
        out += results[c]["outp"].astype(np.float32)
        at = results[c]["attnT"]
        for hl in range(HPC):
            for b in range(B):
                attn[b, c * HPC + hl] = at[hl * 2 + b].astype(np.float32)
    out += bias_total[None, :]
    out = out.reshape(B, S, H)
    return out, attn.transpose(0, 1, 3, 2)


def kernel(_repeat=1, _sim=False, **inputs):
    in_maps, bias_total, S = _prep(inputs)
    if _sim:
        from concourse.bass_interp import CoreSim
        nc = _get_program(S, _repeat)
        results = []
        for c in range(NCORES):
            sim = CoreSim(nc)
            for k2, v2 in in_maps[c].items():
                sim.tensor(k2)[:] = v2
            sim.simulate()
            results.append({"attnT": sim.tensor("attnT").copy(),
                            "outp": sim.tensor("outp").copy()})
    else:
        nc = _get_program(S, _repeat)
        res = bass_utils.run_bass_kernel_spmd(nc, in_maps,
                                              core_ids=list(range(NCORES)))
        results = res.results
    return _assemble(results, bias_total, S)
